# revision 9
# baseline (speedup 1.0000x reference)
"""RNN-T Joiner kernel for 8x TRN2 NeuronCores (Bass/Tile).

out[b,t,u,v] = (enc[b,t]@W_enc.T + b_enc) @ W1.T
            + (pred[b,u]@W_pred.T + b_pred) @ W2.T + b_out
with W1 = W_out[:, :J], W2 = W_out[:, J:].

Strategy: data-parallel over batch (B=8 == n_cores). All biases fold into a
single vector c[v] = W1@b_enc + W2@b_pred + b_out (host-side). Per core:
  S1: E^T[j,t] and P^T[j,u] via PE matmuls (inputs host-transposed and
      pre-packed to [128, X] so each tensor is a single contiguous DMA).
  S2: Ev[t,v] = E@W1.T  (SBUF), Pb[u,v] = P@W2.T + c (K=1 ones-matmul
      folds c into the same PSUM accumulation group).
  S3 (the output): out[t, u, :] = Ev[t, :] + Pb[u, :].
      First UBLK u-values (per t-block) go through the PE: selector-matmul
      broadcasts Pb[u] into PSUM, identity-matmul accumulates Ev, ACT
      copies PSUM -> bf16 out tile. This fills the pipe while Pbrep builds.
      Remaining u: Pb rows are pre-broadcast ("Pbrep" groups of UBLK u,
      double-buffered: sel-matmul -> PSUM -> ACT/GPSIMD copy -> bf16 SBUF);
      the add is then a pure-SBUF bf16 tensor_tensor (DVE 2x mode ~687ns
      per [128,1024] chunk; one chunk per group goes to GPSIMD to keep DVE
      under the DMA roofline).
All matmul operands are bf16 (1 cyc/col on the PE, FWL weight loads);
accumulation stays fp32 in PSUM. Output is written to HBM as bf16 (halves
write traffic; tolerance is 2e-2) and upcast to f32 on host.
"""

import numpy as np

ENC_DIM, DEC_DIM, J, V = 512, 640, 512, 1024
B, T, U = 8, 256, 64
N_CORES = 8
UBLK = 4  # u-values per output tile / DMA ([128, UBLK*1024] bf16 = 1MB DMA)
NG = U // UBLK  # 16 u-groups

_CACHE: dict = {}


def _ensure_path():
    try:
        import concourse.bass  # noqa: F401
    except ImportError:
        import sys

        for p in ("/opt/trn_rl_repo", "/root/.axon_site/_ro/trn_rl_repo"):
            if p not in sys.path:
                sys.path.insert(0, p)


def _build_nc():
    import concourse.mybir as mybir
    from concourse import bacc
    from concourse.masks import make_identity
    from concourse.tile import TileContext

    f32 = mybir.dt.float32
    bf16 = mybir.dt.bfloat16
    nc = bacc.Bacc("TRN2", target_bir_lowering=False, debug=False,
                   num_devices=N_CORES)

    NJ = J // 128   # 4 j-chunks
    NE = ENC_DIM // 128  # 4
    ND = DEC_DIM // 128  # 5
    NT = T // 128   # 2 t-blocks
    NV = V // 512   # 2 psum-bank v-chunks

    # All inputs host-packed to [128, nchunks*cols] (partition-major chunks).
    encT_d = nc.dram_tensor("encT", [128, NE * T], bf16, kind="ExternalInput")
    predT_d = nc.dram_tensor("predT", [128, ND * U], bf16, kind="ExternalInput")
    wencT_d = nc.dram_tensor("w_encT", [128, NE * J], bf16, kind="ExternalInput")
    wpredT_d = nc.dram_tensor("w_predT", [128, ND * J], bf16, kind="ExternalInput")
    w1T_d = nc.dram_tensor("w1T", [128, NJ * V], bf16, kind="ExternalInput")
    w2T_d = nc.dram_tensor("w2T", [128, NJ * V], bf16, kind="ExternalInput")
    cvec_d = nc.dram_tensor("cvec", [1, V], bf16, kind="ExternalInput")
    out_d = nc.dram_tensor("out", [T, U * V], bf16, kind="ExternalOutput")

    with TileContext(nc) as tc:
        with (
            tc.tile_pool(name="const", bufs=1) as const,
            tc.tile_pool(name="pbrep", bufs=2) as pbpool,
            tc.tile_pool(name="otile", bufs=4) as opool,
            tc.tile_pool(name="ps", bufs=4, space="PSUM") as psp,
        ):
            def load(tag, dram, cols):
                t = const.tile([128, cols], bf16, tag=tag, name=tag)
                nc.sync.dma_start(t[:, :], dram.ap()[:, :])
                return t

            wpred_a = load("wpred", wpredT_d, ND * J)
            preds_a = load("pred", predT_d, ND * U)
            wenc_a = load("wenc", wencT_d, NE * J)
            encs_a = load("enc", encT_d, NE * T)
            w2_a = load("w2_", w2T_d, NJ * V)
            w1_a = load("w1_", w1T_d, NJ * V)
            cvec = const.tile([1, V], bf16, tag="cvec", name="cvec")
            nc.sync.dma_start(cvec[:, :], cvec_d.ap()[:, :])

            ident = const.tile([128, 128], bf16, tag="ident", name="ident")
            make_identity(nc, ident[:, :])
            ones = const.tile([1, 128], bf16, tag="ones", name="ones")
            nc.gpsimd.memset(ones[:, :], 1.0)
            # sel[k, u*128+m] = 1 if k == u else 0: sel[:, u*128:(u+1)*128] is
            # the lhsT that broadcasts Pb row u across all 128 out partitions.
            sel = const.tile([U, U * 128], bf16, tag="sel", name="sel")
            nc.gpsimd.memset(sel[:, :], 0.0)
            nc.gpsimd.affine_select(
                out=sel[:, :].rearrange("p (u m) -> p u m", m=128),
                in_=sel[:, :].rearrange("p (u m) -> p u m", m=128),
                compare_op=mybir.AluOpType.not_equal,
                fill=1.0,
                base=0,
                pattern=[[-1, U], [0, 128]],
                channel_multiplier=1,
            )

            # S1b: P^T[j,u] in 4 chunks of [128, 64]
            PT = []
            for m in range(NJ):
                ps = psp.tile([128, V], f32, tag="ps", name="ps")
                for c in range(ND):
                    nc.tensor.matmul(ps[:, :U],
                                     lhsT=wpred_a[:, c * J + m * 128:c * J + (m + 1) * 128],
                                     rhs=preds_a[:, c * U:(c + 1) * U],
                                     start=(c == 0), stop=(c == ND - 1))
                t = const.tile([128, U], bf16, tag=f"PT{m}", name=f"PT{m}")
                nc.scalar.copy(t[:, :], ps[:, :U])
                PT.append(t)

            # S2b: Pb = P @ W2.T + c -> [64, 1024]  (c folded via K=1 matmul)
            Pb = const.tile([U, V], bf16, tag="Pb", name="Pb")
            for vb in range(NV):
                ps = psp.tile([128, V], f32, tag="ps", name="ps")
                for m in range(NJ):
                    nc.tensor.matmul(ps[:U, :512], lhsT=PT[m][:, :],
                                     rhs=w2_a[:, m * V + vb * 512:m * V + (vb + 1) * 512],
                                     start=(m == 0), stop=False)
                nc.tensor.matmul(ps[:U, :512], lhsT=ones[:, :U],
                                 rhs=cvec[:, vb * 512:(vb + 1) * 512],
                                 start=False, stop=True)
                nc.scalar.copy(Pb[:, vb * 512:(vb + 1) * 512], ps[:U, :512])

            # S1a: E^T[j,t] in 4 chunks of [128, 256]
            ET = []
            for m in range(NJ):
                ps = psp.tile([128, V], f32, tag="ps", name="ps")
                for c in range(NE):
                    nc.tensor.matmul(ps[:, :T],
                                     lhsT=wenc_a[:, c * J + m * 128:c * J + (m + 1) * 128],
                                     rhs=encs_a[:, c * T:(c + 1) * T],
                                     start=(c == 0), stop=(c == NE - 1))
                t = const.tile([128, T], bf16, tag=f"ET{m}", name=f"ET{m}")
                nc.scalar.copy(t[:, :], ps[:, :T])
                ET.append(t)

            # S2a: Ev[tb] = E @ W1.T  -> [128, 1024] per t-block
            Ev = [const.tile([128, V], bf16, tag=f"Ev{tb}", name=f"Ev{tb}") for tb in range(NT)]
            for tb in range(NT):
                for vb in range(NV):
                    ps = psp.tile([128, V], f32, tag="ps", name="ps")
                    for m in range(NJ):
                        nc.tensor.matmul(ps[:, :512], lhsT=ET[m][:, tb * 128:(tb + 1) * 128],
                                         rhs=w1_a[:, m * V + vb * 512:m * V + (vb + 1) * 512],
                                         start=(m == 0), stop=(m == NJ - 1))
                    nc.scalar.copy(Ev[tb][:, vb * 512:(vb + 1) * 512], ps[:, :512])

            def build_pbrep(g):
                """Pre-broadcast Pb rows u=g*UBLK..g*UBLK+UBLK-1 across all
                128 partitions: [128, UBLK*1024] bf16. Copies split ACT/GPS."""
                rep = pbpool.tile([128, UBLK * V], bf16, tag="rep", name="rep")
                for uu in range(UBLK):
                    u = g * UBLK + uu
                    ps = psp.tile([128, V], f32, tag="ps", name="ps")
                    for vb in range(NV):
                        sl = slice(vb * 512, (vb + 1) * 512)
                        nc.tensor.matmul(ps[:, sl],
                                         lhsT=sel[:, u * 128:(u + 1) * 128],
                                         rhs=Pb[:, sl],
                                         start=True, stop=True)
                    nc.scalar.copy(rep[:, uu * V:(uu + 1) * V], ps[:, :])
                return rep

            # S3 group 0 via the PE/ACT path (while Pbrep for group 1 builds)
            rep_next = build_pbrep(1)
            for tb in range(NT):
                ot = opool.tile([128, UBLK * V], bf16, tag="ot", name="ot")
                for uu in range(UBLK):
                    u = uu
                    ps = psp.tile([128, V], f32, tag="ps", name="ps")
                    for vb in range(NV):
                        sl = slice(vb * 512, (vb + 1) * 512)
                        nc.tensor.matmul(ps[:, sl],
                                         lhsT=sel[:, u * 128:(u + 1) * 128],
                                         rhs=Pb[:, sl],
                                         start=True, stop=False)
                        nc.tensor.matmul(ps[:, sl],
                                         lhsT=ident[:, :],
                                         rhs=Ev[tb][:, sl],
                                         start=False, stop=True)
                    nc.scalar.copy(ot[:, uu * V:(uu + 1) * V], ps[:, :])
                nc.sync.dma_start(
                    out_d.ap()[tb * 128:(tb + 1) * 128, :UBLK * V], ot[:, :])

            # S3 groups 1..NG-1 via the SBUF-SBUF tensor_tensor path
            for g in range(1, NG):
                rep = rep_next
                if g + 1 < NG:
                    rep_next = build_pbrep(g + 1)
                for tb in range(NT):
                    ot = opool.tile([128, UBLK * V], bf16, tag="ot", name="ot")
                    for uu in range(UBLK):
                        eng = nc.gpsimd if (uu == UBLK - 1 and tb == 1) else nc.vector
                        eng.tensor_tensor(
                            ot[:, uu * V:(uu + 1) * V],
                            Ev[tb][:, :],
                            rep[:, uu * V:(uu + 1) * V],
                            op=mybir.AluOpType.add)
                    nc.sync.dma_start(
                        out_d.ap()[tb * 128:(tb + 1) * 128,
                                   g * UBLK * V:(g + 1) * UBLK * V],
                        ot[:, :])
    nc.compile()
    return nc


def _get_nc():
    if "nc" not in _CACHE:
        _ensure_path()
        _CACHE["nc"] = _build_nc()
    return _CACHE["nc"]


def _pack(a, nchunks):
    """[nchunks*128, C] -> [128, nchunks*C] (chunk-major along columns)."""
    c = a.shape[1]
    return np.ascontiguousarray(
        a.reshape(nchunks, 128, c).transpose(1, 0, 2).reshape(128, nchunks * c))


def _prep_in_maps(enc_out, pred_out, W_enc, b_enc, W_pred, b_pred, W_out, b_out):
    import ml_dtypes

    f = np.float32
    bf = ml_dtypes.bfloat16
    enc_out = np.asarray(enc_out, f)
    pred_out = np.asarray(pred_out, f)
    W_enc = np.asarray(W_enc, f)
    W_pred = np.asarray(W_pred, f)
    W_out = np.asarray(W_out, f)
    W1, W2 = W_out[:, :J], W_out[:, J:]
    cvec = (W1 @ np.asarray(b_enc, f) + W2 @ np.asarray(b_pred, f)
            + np.asarray(b_out, f)).astype(f)[None, :]
    shared = {
        "w_encT": _pack(np.ascontiguousarray(W_enc.T), ENC_DIM // 128).astype(bf),
        "w_predT": _pack(np.ascontiguousarray(W_pred.T), DEC_DIM // 128).astype(bf),
        "w1T": _pack(np.ascontiguousarray(W1.T), J // 128).astype(bf),
        "w2T": _pack(np.ascontiguousarray(W2.T), J // 128).astype(bf),
        "cvec": cvec.astype(bf),
    }
    return [
        {"encT": _pack(np.ascontiguousarray(enc_out[b].T), ENC_DIM // 128).astype(bf),
         "predT": _pack(np.ascontiguousarray(pred_out[b].T), DEC_DIM // 128).astype(bf),
         **shared}
        for b in range(B)
    ]


def run(in_maps, trace=False, **kw):
    _ensure_path()
    from concourse.bass_utils import run_bass_kernel_spmd

    return run_bass_kernel_spmd(_get_nc(), in_maps, list(range(N_CORES)),
                                trace=trace, **kw)


def kernel(enc_out, pred_out, W_enc, b_enc, W_pred, b_pred, W_out, b_out):
    in_maps = _prep_in_maps(enc_out, pred_out, W_enc, b_enc, W_pred, b_pred,
                            W_out, b_out)
    res = run(in_maps, trace=False)
    return np.stack([np.asarray(r["out"]).astype(np.float32).reshape(T, U, V)
                     for r in res.results], axis=0)


# revision 10
# speedup vs baseline: 1.0015x; 1.0015x over previous
"""RNN-T Joiner kernel for 8x TRN2 NeuronCores (Bass/Tile).

out[b,t,u,v] = (enc[b,t]@W_enc.T + b_enc) @ W1.T
            + (pred[b,u]@W_pred.T + b_pred) @ W2.T + b_out
with W1 = W_out[:, :J], W2 = W_out[:, J:].

Strategy: data-parallel over batch (B=8 == n_cores). Host folds the two
back-to-back projections into single matrices (associativity):
  Ev = enc @ (W1@W_enc).T        [T, V]
  Pb = pred @ (W2@W_pred).T + c  [U, V],  c = W1@b_enc + W2@b_pred + b_out
so the device does one GEMM stage instead of two. Per core (one batch):
  S2: Ev (2 t-blocks of 128) and Pb via PE matmuls into PSUM, ACT copies
      to bf16 SBUF. Inputs host-transposed and pre-packed to [128, X] so
      each tensor is a single contiguous DMA.
  S3 (the output): out[t, u, :] = Ev[t, :] + Pb[u, :].
      First UBLK u-values (per t-block) go through the PE: selector-matmul
      broadcasts Pb[u] into PSUM, identity-matmul accumulates Ev, ACT
      copies PSUM -> bf16 out tile. This fills the pipe while Pbrep builds.
      Remaining u: Pb rows are pre-broadcast ("Pbrep" groups of UBLK u,
      double-buffered: sel-matmul -> PSUM -> ACT copy -> bf16 SBUF); the
      add is then ONE pure-SBUF bf16 DVE tensor_tensor per out tile
      (FD=4096, in0 = Ev repeated via a stride-0 broadcast AP, 2x mode).
All matmul operands are bf16 (1 cyc/col on the PE, FWL weight loads);
accumulation stays fp32 in PSUM. Output is written to HBM as bf16 (halves
write traffic; tolerance is 2e-2) and upcast to f32 on host.
"""

import numpy as np

ENC_DIM, DEC_DIM, J, V = 512, 640, 512, 1024
B, T, U = 8, 256, 64
N_CORES = 8
UBLK = 4  # u-values per output tile / DMA ([128, UBLK*1024] bf16 = 1MB DMA)
NG = U // UBLK  # 16 u-groups

_CACHE: dict = {}


def _ensure_path():
    try:
        import concourse.bass  # noqa: F401
    except ImportError:
        import sys

        for p in ("/opt/trn_rl_repo", "/root/.axon_site/_ro/trn_rl_repo"):
            if p not in sys.path:
                sys.path.insert(0, p)


def _build_nc():
    import concourse.mybir as mybir
    from concourse import bacc
    from concourse.masks import make_identity
    from concourse.tile import TileContext

    f32 = mybir.dt.float32
    bf16 = mybir.dt.bfloat16
    nc = bacc.Bacc("TRN2", target_bir_lowering=False, debug=False,
                   num_devices=N_CORES)

    NE = ENC_DIM // 128  # 4 contraction chunks for Ev
    ND = DEC_DIM // 128  # 5 contraction chunks for Pb
    NT = T // 128   # 2 t-blocks
    NV = V // 512   # 2 psum-bank v-chunks

    # All inputs host-packed to [128, nchunks*cols] (partition-major chunks).
    encT_d = nc.dram_tensor("encT", [128, NE * T], bf16, kind="ExternalInput")
    predT_d = nc.dram_tensor("predT", [128, ND * U], bf16, kind="ExternalInput")
    wce_d = nc.dram_tensor("wceT", [128, NE * V], bf16, kind="ExternalInput")
    wcp_d = nc.dram_tensor("wcpT", [128, ND * V], bf16, kind="ExternalInput")
    cvec_d = nc.dram_tensor("cvec", [1, V], bf16, kind="ExternalInput")
    out_d = nc.dram_tensor("out", [T, U * V], bf16, kind="ExternalOutput")

    with TileContext(nc) as tc:
        with (
            tc.tile_pool(name="const", bufs=1) as const,
            tc.tile_pool(name="pbrep", bufs=2) as pbpool,
            tc.tile_pool(name="otile", bufs=4) as opool,
            tc.tile_pool(name="ps", bufs=4, space="PSUM") as psp,
        ):
            def load(tag, dram, cols):
                t = const.tile([128, cols], bf16, tag=tag, name=tag)
                nc.sync.dma_start(t[:, :], dram.ap()[:, :])
                return t

            preds_a = load("pred", predT_d, ND * U)
            wcp_a = load("wcp", wcp_d, ND * V)
            encs_a = load("enc", encT_d, NE * T)
            wce_a = load("wce", wce_d, NE * V)
            cvec = const.tile([1, V], bf16, tag="cvec", name="cvec")
            nc.sync.dma_start(cvec[:, :], cvec_d.ap()[:, :])

            ident = const.tile([128, 128], bf16, tag="ident", name="ident")
            make_identity(nc, ident[:, :])
            ones = const.tile([1, 128], bf16, tag="ones", name="ones")
            nc.gpsimd.memset(ones[:, :], 1.0)
            # sel[k, u*128+m] = 1 if k == u else 0: sel[:, u*128:(u+1)*128] is
            # the lhsT that broadcasts Pb row u across all 128 out partitions.
            sel = const.tile([U, U * 128], bf16, tag="sel", name="sel")
            nc.gpsimd.memset(sel[:, :], 0.0)
            nc.gpsimd.affine_select(
                out=sel[:, :].rearrange("p (u m) -> p u m", m=128),
                in_=sel[:, :].rearrange("p (u m) -> p u m", m=128),
                compare_op=mybir.AluOpType.not_equal,
                fill=1.0,
                base=0,
                pattern=[[-1, U], [0, 128]],
                channel_multiplier=1,
            )

            # S2b: Pb = pred @ Wcp.T + c -> [64, 1024]  (c via K=1 matmul)
            Pb = const.tile([U, V], bf16, tag="Pb", name="Pb")
            for vb in range(NV):
                ps = psp.tile([128, V], f32, tag="ps", name="ps")
                for c in range(ND):
                    nc.tensor.matmul(ps[:U, :512],
                                     lhsT=preds_a[:, c * U:(c + 1) * U],
                                     rhs=wcp_a[:, c * V + vb * 512:c * V + (vb + 1) * 512],
                                     start=(c == 0), stop=False)
                nc.tensor.matmul(ps[:U, :512], lhsT=ones[:, :U],
                                 rhs=cvec[:, vb * 512:(vb + 1) * 512],
                                 start=False, stop=True)
                nc.scalar.copy(Pb[:, vb * 512:(vb + 1) * 512], ps[:U, :512])

            # S2a: Ev[tb] = enc @ Wce.T -> [128, 1024] per t-block
            Ev = [const.tile([128, V], bf16, tag=f"Ev{tb}", name=f"Ev{tb}") for tb in range(NT)]
            for tb in range(NT):
                for vb in range(NV):
                    ps = psp.tile([128, V], f32, tag="ps", name="ps")
                    for c in range(NE):
                        nc.tensor.matmul(
                            ps[:, :512],
                            lhsT=encs_a[:, c * T + tb * 128:c * T + (tb + 1) * 128],
                            rhs=wce_a[:, c * V + vb * 512:c * V + (vb + 1) * 512],
                            start=(c == 0), stop=(c == NE - 1))
                    nc.scalar.copy(Ev[tb][:, vb * 512:(vb + 1) * 512], ps[:, :512])

            def build_pbrep(g):
                """Pre-broadcast Pb rows u=g*UBLK..g*UBLK+UBLK-1 across all
                128 partitions: [128, UBLK*1024] bf16."""
                rep = pbpool.tile([128, UBLK * V], bf16, tag="rep", name="rep")
                for uu in range(UBLK):
                    u = g * UBLK + uu
                    ps = psp.tile([128, V], f32, tag="ps", name="ps")
                    for vb in range(NV):
                        sl = slice(vb * 512, (vb + 1) * 512)
                        nc.tensor.matmul(ps[:, sl],
                                         lhsT=sel[:, u * 128:(u + 1) * 128],
                                         rhs=Pb[:, sl],
                                         start=True, stop=True)
                    nc.scalar.copy(rep[:, uu * V:(uu + 1) * V], ps[:, :])
                return rep

            # S3 group 0 via the PE/ACT path (while Pbrep for group 1 builds)
            rep_next = build_pbrep(1)
            for tb in range(NT):
                ot = opool.tile([128, UBLK * V], bf16, tag="ot", name="ot")
                for uu in range(UBLK):
                    u = uu
                    ps = psp.tile([128, V], f32, tag="ps", name="ps")
                    for vb in range(NV):
                        sl = slice(vb * 512, (vb + 1) * 512)
                        nc.tensor.matmul(ps[:, sl],
                                         lhsT=sel[:, u * 128:(u + 1) * 128],
                                         rhs=Pb[:, sl],
                                         start=True, stop=False)
                        nc.tensor.matmul(ps[:, sl],
                                         lhsT=ident[:, :],
                                         rhs=Ev[tb][:, sl],
                                         start=False, stop=True)
                    nc.scalar.copy(ot[:, uu * V:(uu + 1) * V], ps[:, :])
                nc.sync.dma_start(
                    out_d.ap()[tb * 128:(tb + 1) * 128, :UBLK * V], ot[:, :])

            # S3 groups 1..NG-1: one FD=4096 DVE tensor_tensor per out tile
            # (in0 = Ev repeated UBLK times via stride-0 broadcast AP).
            for g in range(1, NG):
                rep = rep_next
                if g + 1 < NG:
                    rep_next = build_pbrep(g + 1)
                for tb in range(NT):
                    ot = opool.tile([128, UBLK * V], bf16, tag="ot", name="ot")
                    nc.vector.tensor_tensor(
                        ot[:, :].rearrange("p (r v) -> p r v", v=V),
                        Ev[tb][:, :].unsqueeze(1).broadcast_to((128, UBLK, V)),
                        rep[:, :].rearrange("p (r v) -> p r v", v=V),
                        op=mybir.AluOpType.add)
                    nc.sync.dma_start(
                        out_d.ap()[tb * 128:(tb + 1) * 128,
                                   g * UBLK * V:(g + 1) * UBLK * V],
                        ot[:, :])
    nc.compile()
    return nc


def _get_nc():
    if "nc" not in _CACHE:
        _ensure_path()
        _CACHE["nc"] = _build_nc()
    return _CACHE["nc"]


def _pack(a, nchunks):
    """[nchunks*128, C] -> [128, nchunks*C] (chunk-major along columns)."""
    c = a.shape[1]
    return np.ascontiguousarray(
        a.reshape(nchunks, 128, c).transpose(1, 0, 2).reshape(128, nchunks * c))


def _prep_in_maps(enc_out, pred_out, W_enc, b_enc, W_pred, b_pred, W_out, b_out):
    import ml_dtypes

    f = np.float32
    bf = ml_dtypes.bfloat16
    enc_out = np.asarray(enc_out, f)
    pred_out = np.asarray(pred_out, f)
    W_enc = np.asarray(W_enc, f)
    W_pred = np.asarray(W_pred, f)
    W_out = np.asarray(W_out, f)
    W1, W2 = W_out[:, :J], W_out[:, J:]
    cvec = (W1 @ np.asarray(b_enc, f) + W2 @ np.asarray(b_pred, f)
            + np.asarray(b_out, f)).astype(f)[None, :]
    wce = W1 @ W_enc    # [V, ENC_DIM]
    wcp = W2 @ W_pred   # [V, DEC_DIM]
    shared = {
        "wceT": _pack(np.ascontiguousarray(wce.T), ENC_DIM // 128).astype(bf),
        "wcpT": _pack(np.ascontiguousarray(wcp.T), DEC_DIM // 128).astype(bf),
        "cvec": cvec.astype(bf),
    }
    return [
        {"encT": _pack(np.ascontiguousarray(enc_out[b].T), ENC_DIM // 128).astype(bf),
         "predT": _pack(np.ascontiguousarray(pred_out[b].T), DEC_DIM // 128).astype(bf),
         **shared}
        for b in range(B)
    ]


def run(in_maps, trace=False, **kw):
    _ensure_path()
    from concourse.bass_utils import run_bass_kernel_spmd

    return run_bass_kernel_spmd(_get_nc(), in_maps, list(range(N_CORES)),
                                trace=trace, **kw)


def kernel(enc_out, pred_out, W_enc, b_enc, W_pred, b_pred, W_out, b_out):
    in_maps = _prep_in_maps(enc_out, pred_out, W_enc, b_enc, W_pred, b_pred,
                            W_out, b_out)
    res = run(in_maps, trace=False)
    return np.stack([np.asarray(r["out"]).astype(np.float32).reshape(T, U, V)
                     for r in res.results], axis=0)
